# revision 9
# baseline (speedup 1.0000x reference)
"""MultiHeadAttention (causal + ALiBi) Trainium2 kernel, 8-core SPMD.

Sharding: core c -> batch b = c // 4, head-group j = c % 4 owning global
heads {j, j+4, j+8, j+12} (strided so every core gets one head from each
slope class). Each core projects q/k/v for its 4 heads from x[b], runs
windowed-causal attention in a transposed layout (scores^T[j_kv, i_q]),
and emits a partial out-projection [S, D]. Host sums the 4 partials per
batch (the "all-reduce") and returns [B, S, D].

Math notes:
- ALiBi bias slope*(j-i): the -slope*i part is constant per softmax row
  and cancels; the slope*j part is per-partition in the scores^T layout
  and rides the ACT exp bias input. Blocks are re-centered per i-chunk
  (bias slope*(j - M_it)) to bound exp's dynamic range; the common
  factor cancels in num/l.
- Head slots use per-slot i-chunk widths W (64 for the steepest heads,
  256 otherwise) so that slope*(W-1) stays within fp32's exp range, and
  a j-window (ALiBi locality) skips blocks with negligible weight.
- Matmuls run in float32r (full-rate fp32 variant, ~1.5e-4 rel err) for
  W>=256 and plain fp32 for the W=64 slot (same cost at N<256).
"""
import math
from contextlib import ExitStack

import numpy as np

import concourse.bass as bass
import concourse.tile as tile
from concourse import bacc, mybir
from concourse.bass_utils import run_bass_kernel_spmd

B, S, D, H, HD = 2, 2048, 1024, 16, 64
N_CORES = 8
DT = mybir.dt
F32, F32R = DT.float32, DT.float32r
NEG = -1.0e30

SLOT_W = [64, 256, 256, 256]           # i-chunk width per head slot
SLOT_WIN = [248, 992, 10 ** 9, 10 ** 9]  # j-window per slot (uniform = max over slot heads)
SLOT_DT = [F32, F32R, F32R, F32R]      # matmul dtype for scores/PV per slot


def slot_blocks(slot):
    """(it, jt, o) list, uniform across cores. o = i0 - 128*jt."""
    W, win = SLOT_W[slot], SLOT_WIN[slot]
    blocks = []
    for it in range(S // W):
        i0 = it * W
        jt_max = (i0 + W - 1) // 128
        jt_min = max(0, math.ceil((i0 - win - 127) / 128))
        for jt in range(jt_min, jt_max + 1):
            blocks.append((it, jt, i0 - 128 * jt))
    return blocks


def slot_offsets(slot):
    """Sorted distinct o values for a slot (bias tile index space)."""
    return sorted({o for _, _, o in slot_blocks(slot)})


def build_nc(repeat=1):
    nc = bacc.Bacc(
        "TRN2", target_bir_lowering=False, debug=False,
        enable_asserts=False, num_devices=N_CORES,
    )
    dram = {}

    def din(name, shape, dtype):
        dram[name] = nc.dram_tensor(name, shape, dtype, kind="ExternalInput").ap()
        return dram[name]

    xT = din("xT", [D, S], F32R)
    wqT = din("wqT", [D, 256], F32R)
    wkT = din("wkT", [D, 256], F32R)
    wvT = din("wvT", [D, 256], F32R)
    bq_row = din("bq_row", [1, 256], F32R)
    bk_row = din("bk_row", [1, 256], F32R)
    ones_row = din("ones_row", [1, 512], F32R)
    ones_col_r = din("ones_col_r", [128, 1], F32R)
    ones_col_f = din("ones_col_f", [128, 1], F32)
    masks_a = din("masks_a", [2, 128, 64], F32)
    masks_b = din("masks_b", [2, 128, 256], F32)
    nb = [len(slot_offsets(s)) for s in range(4)]
    bias_in = [din(f"bias{s}", [nb[s], 128, 1], F32) for s in range(4)]
    wout_ab = din("wout_ab", [128, D], F32R)
    wout_cd = din("wout_cd", [128, D], F32R)
    yconst = din("yconst", [1, D], F32)
    y_out = nc.dram_tensor("y", [S, D], F32, kind="ExternalOutput").ap()

    with tile.TileContext(nc) as tc:
        for _ in range(repeat):
            build_body(tc, dram, y_out)
    nc.compile()
    return nc


def build_body(tc, dram, y_out):
    nc = tc.nc
    Exp = mybir.ActivationFunctionType.Exp
    with ExitStack() as ctx:
        consts = ctx.enter_context(tc.tile_pool(name="consts", bufs=1))
        qkpool = ctx.enter_context(tc.tile_pool(name="qk", bufs=1))
        vpool = ctx.enter_context(tc.tile_pool(name="vp", bufs=1))
        attnp = ctx.enter_context(tc.tile_pool(name="attn", bufs=1))

        # ---- constants ----
        mask_a_sb, mask_b_sb = [], []
        for mi in range(2):
            t = consts.tile([128, 64], F32, tag=f"maska{mi}", name=f"maska{mi}")
            nc.sync.dma_start(out=t[:], in_=dram["masks_a"][mi])
            mask_a_sb.append(t)
            t = consts.tile([128, 256], F32, tag=f"maskb{mi}", name=f"maskb{mi}")
            nc.sync.dma_start(out=t[:], in_=dram["masks_b"][mi])
            mask_b_sb.append(t)
        bias_sb = []
        for s in range(4):
            offs = slot_offsets(s)
            tiles = []
            for i in range(len(offs)):
                t = consts.tile([128, 1], F32, tag=f"bias{s}_{i}", name=f"bias{s}_{i}")
                nc.sync.dma_start(out=t[:], in_=dram[f"bias{s}"][i])
                tiles.append(t)
            bias_sb.append(dict(zip(offs, tiles)))
        wout_sb = []
        for nm in ("wout_ab", "wout_cd"):
            t = consts.tile([128, D], F32R, tag=nm, name=nm)
            nc.sync.dma_start(out=t[:], in_=dram[nm])
            wout_sb.append(t)
        yconst_sb = consts.tile([1, D], F32, tag="yconst", name="yconst")
        nc.sync.dma_start(out=yconst_sb[:], in_=dram["yconst"])
        yconst_bc = consts.tile([128, D], F32, tag="yconst_bc", name="yconst_bc")
        nc.gpsimd.partition_broadcast(yconst_bc[:], yconst_sb[:])

        # ---- persistent q/k/v/attn tiles ----
        q_t = [qkpool.tile([64, S], SLOT_DT[s], tag=f"q{s}", name=f"q{s}") for s in range(4)]
        k_t = [qkpool.tile([64, S], SLOT_DT[s], tag=f"k{s}", name=f"k{s}") for s in range(4)]
        # V' [128, 16, 65]: per j-tile 64 value cols + ones col
        v_t = [vpool.tile([128, 16, 65], SLOT_DT[s], tag=f"v{s}", name=f"v{s}") for s in range(4)]
        attn_sb = [attnp.tile([128, S], F32R, tag=f"attn{i}", name=f"attn{i}") for i in range(2)]

        # ---- phase A: QKV projection ----
        with ExitStack() as pa:
            xtp = pa.enter_context(tc.tile_pool(name="xt", bufs=16))
            wp = pa.enter_context(tc.tile_pool(name="w", bufs=8))
            rowp = pa.enter_context(tc.tile_pool(name="rows", bufs=1))
            qkv_ps = pa.enter_context(tc.tile_pool(name="qkv_ps", bufs=3, space="PSUM"))

            w_sb = {}
            for nm, dr in (("q", "wqT"), ("k", "wkT"), ("v", "wvT")):
                w_sb[nm] = []
                for kt in range(8):
                    t = wp.tile([128, 256], F32R, tag=f"w{nm}", name=f"w{nm}")
                    nc.sync.dma_start(
                        out=t[:], in_=dram[dr][kt * 128:(kt + 1) * 128, :])
                    w_sb[nm].append(t)
            brow = {}
            for nm, dr in (("q", "bq_row"), ("k", "bk_row")):
                t = rowp.tile([1, 256], F32R, tag=f"b{nm}", name=f"b{nm}")
                nc.sync.dma_start(out=t[:], in_=dram[dr])
                brow[nm] = t
            ones_row_sb = rowp.tile([1, 512], F32R, tag="ones_row", name="ones_row")
            nc.sync.dma_start(out=ones_row_sb[:], in_=dram["ones_row"])

            # stream x^T by 512-token chunks; q/k/v projections per chunk
            for ch in range(4):
                xt = []
                for kt in range(8):
                    t = xtp.tile([128, 512], F32R, tag="xt", name="xt")
                    nc.sync.dma_start(
                        out=t[:],
                        in_=dram["xT"][kt * 128:(kt + 1) * 128,
                                       ch * 512:(ch + 1) * 512])
                    xt.append(t)
                sl = slice(ch * 512, (ch + 1) * 512)
                for nm, dst in (("q", q_t), ("k", k_t)):
                    for ft in range(2):      # feature pair (slots 2ft, 2ft+1)
                        ps = qkv_ps.tile([128, 512], F32, tag="qkv", name="qkv")
                        for kt in range(8):
                            nc.tensor.matmul(
                                ps[:], w_sb[nm][kt][:, ft * 128:(ft + 1) * 128],
                                xt[kt][:], start=(kt == 0), stop=False)
                        nc.tensor.matmul(
                            ps[:], brow[nm][0:1, ft * 128:(ft + 1) * 128],
                            ones_row_sb[:], start=False, stop=True)
                        nc.vector.tensor_copy(dst[2 * ft][0:64, sl], ps[0:64, :])
                        nc.vector.tensor_copy(dst[2 * ft + 1][0:64, sl], ps[64:128, :])
                # v projection -> v_t (natural layout [token, feat])
                for tl in range(4):
                    tt = ch * 4 + tl
                    ps = qkv_ps.tile([128, 256], F32, tag="qkv", name="qkvv")
                    for kt in range(8):
                        nc.tensor.matmul(
                            ps[:], xt[kt][:, tl * 128:(tl + 1) * 128],
                            w_sb["v"][kt][:], start=(kt == 0), stop=(kt == 7))
                    for s in range(4):
                        nc.vector.tensor_copy(
                            v_t[s][:, tt:tt + 1, 0:64], ps[:, s * 64:(s + 1) * 64])
            # ones columns of V'
            for s in range(4):
                ones_dram = dram["ones_col_f" if SLOT_DT[s] == F32 else "ones_col_r"]
                src = bass.AP(
                    tensor=ones_dram.tensor, offset=0,
                    ap=[[1, 128], [0, 16], [0, 1]])
                nc.sync.dma_start(out=v_t[s][:, :, 64:65], in_=src)

        # ---- phase B: attention per slot ----
        with ExitStack() as pb:
            sc_ps = pb.enter_context(tc.tile_pool(name="sc_ps", bufs=3, space="PSUM"))
            pv_ps = pb.enter_context(tc.tile_pool(name="pv_ps", bufs=2, space="PSUM"))
            prp = pb.enter_context(tc.tile_pool(name="probs", bufs=4))
            lp = pb.enter_context(tc.tile_pool(name="lvec", bufs=4))
            rbp = pb.enter_context(tc.tile_pool(name="rbc", bufs=3))

            for s in range(4):
                W = SLOT_W[s]
                dt_s = SLOT_DT[s]
                blocks = slot_blocks(s)
                by_it = {}
                for it, jt, o in blocks:
                    by_it.setdefault(it, []).append((jt, o))
                for it in range(S // W):
                    blist = by_it[it]
                    pv = pv_ps.tile([65, W], F32, tag="pv", name="pv")
                    for bi, (jt, o) in enumerate(blist):
                        sc = sc_ps.tile([128, W], F32, tag="sc", name="sc")
                        nc.tensor.matmul(
                            sc[:], k_t[s][:, jt * 128:(jt + 1) * 128],
                            q_t[s][:, it * W:(it + 1) * W],
                            start=True, stop=True)
                        if o <= 127:  # diagonal block -> causal mask add
                            if s == 0:
                                msk = mask_a_sb[o // 64]
                            else:
                                msk = mask_b_sb[0 if o == 0 else 1]
                            nc.vector.tensor_add(sc[:], sc[:], msk[:])
                        pr = prp.tile([128, W], dt_s, tag="pr", name="pr")
                        nc.scalar.activation(pr[:], sc[:], Exp, bias=bias_sb[s][o][:])
                        nc.tensor.matmul(
                            pv[:], v_t[s][:, jt:jt + 1, :], pr[:],
                            start=(bi == 0), stop=(bi == len(blist) - 1))
                    # epilogue: l -> 1/l -> broadcast -> normalize into attn tile
                    ln = lp.tile([1, W], F32, tag="ln", name="ln")
                    nc.vector.tensor_copy(ln[:], pv[64:65, :])
                    rr = lp.tile([1, W], F32, tag="rr", name="rr")
                    nc.vector.reciprocal(rr[:], ln[:])
                    rb = rbp.tile([64, W], F32, tag="rb", name="rb")
                    nc.gpsimd.partition_broadcast(rb[:], rr[:])
                    dst = attn_sb[s // 2]
                    r0 = (s % 2) * 64
                    nc.vector.tensor_mul(
                        dst[r0:r0 + 64, it * W:(it + 1) * W], pv[0:64, :], rb[:])

        # ---- phase C: out projection ----
        with ExitStack() as pc:
            y_ps = pc.enter_context(tc.tile_pool(name="y_ps", bufs=2, space="PSUM"))
            yp = pc.enter_context(tc.tile_pool(name="ysb", bufs=3))
            for tt in range(16):
                for oc in range(2):
                    py = y_ps.tile([128, 512], F32, tag="py", name="py")
                    nc.tensor.matmul(
                        py[:], attn_sb[0][:, tt * 128:(tt + 1) * 128],
                        wout_sb[0][:, oc * 512:(oc + 1) * 512],
                        start=True, stop=False)
                    nc.tensor.matmul(
                        py[:], attn_sb[1][:, tt * 128:(tt + 1) * 128],
                        wout_sb[1][:, oc * 512:(oc + 1) * 512],
                        start=False, stop=True)
                    ysb = yp.tile([128, 512], F32, tag="ysb", name="ysb")
                    nc.vector.tensor_add(ysb[:], py[:], yconst_bc[:, oc * 512:(oc + 1) * 512])
                    nc.sync.dma_start(
                        out=y_out[tt * 128:(tt + 1) * 128, oc * 512:(oc + 1) * 512],
                        in_=ysb[:])


def make_in_maps(x, w_qkv, b_qkv, w_out, b_out):
    """Host-side sharding + constant prep. Returns list of 8 in_maps."""
    x = np.asarray(x, np.float32)
    w_qkv = np.asarray(w_qkv, np.float32)
    b_qkv = np.asarray(b_qkv, np.float32)
    w_out = np.asarray(w_out, np.float32)
    b_out = np.asarray(b_out, np.float32)

    slopes = (2.0 ** (-(np.arange(1, H + 1)) * 8.0 / H)).astype(np.float64)

    # shared constants
    ones_row_np = np.ones((1, 512), np.float32)
    ones_col = np.ones((128, 1), np.float32)
    masks_a = np.empty((2, 128, 64), np.float32)
    for mi, o in enumerate((0, 64)):
        p = np.arange(128)[:, None]
        f = np.arange(64)[None, :]
        masks_a[mi] = np.where(p <= o + f, 0.0, NEG)
    masks_b = np.empty((2, 128, 256), np.float32)
    for mi, o in enumerate((0, -128)):
        p = np.arange(128)[:, None]
        f = np.arange(256)[None, :]
        masks_b[mi] = np.where(p <= o + f, 0.0, NEG)

    in_maps = []
    for c in range(N_CORES):
        b, j = divmod(c, 4)
        heads = [j, j + 4, j + 8, j + 12]
        cols = np.concatenate([np.arange(h * HD, (h + 1) * HD) for h in heads])
        wq = w_qkv[cols, :] / 8.0                  # [256, 1024], scale folded
        wk = w_qkv[D + cols, :]
        wv = w_qkv[2 * D + cols, :]
        bq = b_qkv[cols] / 8.0
        bk = b_qkv[D + cols]
        bv = b_qkv[2 * D + cols]
        w_out_loc = w_out[:, cols]                  # [1024, 256]
        yconst = (w_out_loc @ bv + b_out / 4.0).astype(np.float32)[None, :]

        biases = {}
        for s in range(4):
            offs = slot_offsets(s)
            Wl = SLOT_W[s]
            sl = slopes[heads[s]]
            arr = np.empty((len(offs), 128, 1), np.float32)
            for i, o in enumerate(offs):
                arr[i, :, 0] = sl * (np.arange(128) - o - Wl + 1)
            biases[f"bias{s}"] = arr

        in_maps.append(dict(
            xT=np.ascontiguousarray(x[b].T),
            wqT=np.ascontiguousarray(wq.T),
            wkT=np.ascontiguousarray(wk.T),
            wvT=np.ascontiguousarray(wv.T),
            bq_row=np.ascontiguousarray(bq[None, :]),
            bk_row=np.ascontiguousarray(bk[None, :]),
            ones_row=ones_row_np, ones_col_r=ones_col, ones_col_f=ones_col,
            masks_a=masks_a, masks_b=masks_b,
            wout_ab=np.ascontiguousarray(w_out_loc[:, 0:128].T),
            wout_cd=np.ascontiguousarray(w_out_loc[:, 128:256].T),
            yconst=yconst,
            **biases,
        ))
    return in_maps


_NC_CACHE = {}


def _get_nc(repeat=1):
    if repeat not in _NC_CACHE:
        _NC_CACHE[repeat] = build_nc(repeat)
    return _NC_CACHE[repeat]


def kernel(x, w_qkv, b_qkv, w_out, b_out, block_mask=None):
    in_maps = make_in_maps(x, w_qkv, b_qkv, w_out, b_out)
    nc = _get_nc(1)
    res = run_bass_kernel_spmd(nc, in_maps, list(range(N_CORES)), trace=False)
    y = np.zeros((B, S, D), np.float64)
    for c in range(N_CORES):
        y[c // 4] += res.results[c]["y"].astype(np.float64)
    return y.astype(np.float32)


# revision 22
# speedup vs baseline: 850.0151x; 850.0151x over previous
"""MultiHeadAttention (causal + ALiBi) Trainium2 kernel, 8-core SPMD.

Sharding: core c -> batch b = c // 4, head-group j = c % 4 owning global
heads {j, j+4, j+8, j+12} (strided so every core gets one head from each
slope class). Each core projects q/k/v for its 4 heads from x[b], runs
windowed-causal attention in a transposed layout (scores^T[j_kv, i_q]),
and emits a partial out-projection [S, D]. Host sums the 4 partials per
batch (the "all-reduce") and returns [B, S, D].

Math notes:
- ALiBi bias slope*(j-i): the -slope*i part is constant per softmax row
  and cancels; the slope*j part is per-partition in the scores^T layout
  and rides the ACT exp bias input. Blocks are re-centered per i-chunk
  (bias slope*(j - M_it)) to bound exp's dynamic range; the common
  factor cancels in num/l.
- Head slots use per-slot i-chunk widths W (64 for the steepest heads,
  256 otherwise) so that slope*(W-1) stays within fp32's exp range, and
  a j-window (ALiBi locality) skips blocks with negligible weight.
- Matmuls run in float32r (full-rate fp32 variant, ~1.5e-4 rel err) for
  W>=256 and plain fp32 for the W=64 slot (same cost at N<256).
"""
import math
from contextlib import ExitStack

import numpy as np

import concourse.bass as bass
import concourse.tile as tile
from concourse import bacc, mybir
from concourse.bass_utils import run_bass_kernel_spmd

B, S, D, H, HD = 2, 2048, 1024, 16, 64
N_CORES = 8
DT = mybir.dt
F32, F32R = DT.float32, DT.float32r
NEG = -1.0e30

SLOT_W = [64, 256, 256, 256]           # i-chunk width per head slot
SLOT_WIN = [248, 992, 10 ** 9, 10 ** 9]  # j-window per slot (uniform = max over slot heads)
SLOT_DT = [F32R, F32R, F32R, F32R]     # matmul dtype for scores/PV per slot


def slot_blocks(slot):
    """(it, jt, o) list, uniform across cores. o = i0 - 128*jt."""
    W, win = SLOT_W[slot], SLOT_WIN[slot]
    blocks = []
    for it in range(S // W):
        i0 = it * W
        jt_max = (i0 + W - 1) // 128
        jt_min = max(0, math.ceil((i0 - win - 127) / 128))
        for jt in range(jt_min, jt_max + 1):
            blocks.append((it, jt, i0 - 128 * jt))
    return blocks


def slot_offsets(slot):
    """Sorted distinct o values for a slot (bias tile index space)."""
    return sorted({o for _, _, o in slot_blocks(slot)})


def build_nc(repeat=1):
    nc = bacc.Bacc(
        "TRN2", target_bir_lowering=False, debug=False,
        enable_asserts=False, num_devices=N_CORES,
    )
    dram = {}

    def din(name, shape, dtype):
        dram[name] = nc.dram_tensor(name, shape, dtype, kind="ExternalInput").ap()
        return dram[name]

    xT = din("xT", [D, S], F32R)
    wqT = din("wqT", [D, 256], F32R)
    wkT = din("wkT", [D, 256], F32R)
    wvT = din("wvT", [D, 256], F32R)
    bq_p = din("bq_p", [2, 128, 1], F32)
    bk_p = din("bk_p", [2, 128, 1], F32)
    masks_a = din("masks_a", [2, 128, 64], F32)
    masks_b = din("masks_b", [2, 128, 256], F32)
    nbtot = sum(len(slot_offsets(s)) for s in range(4))
    bias_all = din("bias_all", [128, nbtot], F32)
    wout_ab = din("wout_ab", [128, D], F32R)
    wout_cd = din("wout_cd", [128, D], F32R)
    yconst = din("yconst", [1, D], F32)
    y_out = nc.dram_tensor("y", [S, D], F32, kind="ExternalOutput").ap()

    with tile.TileContext(nc) as tc:
        for _ in range(repeat):
            build_body(tc, dram, y_out)
    nc.compile()
    return nc


def build_body(tc, dram, y_out):
    nc = tc.nc
    Exp = mybir.ActivationFunctionType.Exp
    with ExitStack() as ctx:
        consts = ctx.enter_context(tc.tile_pool(name="consts", bufs=1))
        qkpool = ctx.enter_context(tc.tile_pool(name="qk", bufs=1))
        vpool = ctx.enter_context(tc.tile_pool(name="vp", bufs=1))
        attnp = ctx.enter_context(tc.tile_pool(name="attn", bufs=1))
        xtp = ctx.enter_context(tc.tile_pool(name="xt", bufs=12))
        wp = ctx.enter_context(tc.tile_pool(name="w", bufs=8))
        rowp = ctx.enter_context(tc.tile_pool(name="rows", bufs=1))
        prp = ctx.enter_context(tc.tile_pool(name="probs", bufs=20))
        lp = ctx.enter_context(tc.tile_pool(name="lvec", bufs=4))
        rbp = ctx.enter_context(tc.tile_pool(name="rbc", bufs=3))
        yp = ctx.enter_context(tc.tile_pool(name="ysb", bufs=4))
        # PSUM budget (8 banks): big(qkv+y)=3, sc=3, pv=2
        big_ps = ctx.enter_context(tc.tile_pool(name="big_ps", bufs=3, space="PSUM"))
        sc_ps = ctx.enter_context(tc.tile_pool(name="sc_ps", bufs=3, space="PSUM"))
        pv_ps = ctx.enter_context(tc.tile_pool(name="pv_ps", bufs=2, space="PSUM"))

        # ---- persistent q/k/v/attn tiles ----
        # q/k stored as slot-pair tiles [128, S]: slot s lives in partition
        # half (s % 2) of pair tile s // 2
        q_p = [qkpool.tile([128, S], F32R, tag=f"qp{i}", name=f"qp{i}") for i in range(2)]
        k_p = [qkpool.tile([128, S], F32R, tag=f"kp{i}", name=f"kp{i}") for i in range(2)]
        # V' [128, 16, 65]: per j-tile 64 value cols + ones col
        v_t = [vpool.tile([128, 16, 65], SLOT_DT[s], tag=f"v{s}", name=f"v{s}") for s in range(4)]
        attn_sb = [attnp.tile([128, S], F32R, tag=f"attn{i}", name=f"attn{i}") for i in range(2)]

        # ---- phase-A weights (DMA-emitted first: on the startup critical path)
        w_sb = {}
        for nm, dr, eng in (("q", "wqT", nc.sync), ("k", "wkT", nc.gpsimd),
                            ("v", "wvT", nc.gpsimd)):
            w_sb[nm] = []
            for kt in range(8):
                t = wp.tile([128, 256], F32R, tag=f"w{nm}", name=f"w{nm}")
                eng.dma_start(out=t[:], in_=dram[dr][kt * 128:(kt + 1) * 128, :])
                w_sb[nm].append(t)
        bpair = {}
        for nm, dr in (("q", "bq_p"), ("k", "bk_p")):
            tiles = []
            for ft in range(2):
                t = rowp.tile([128, 1], F32, tag=f"b{nm}{ft}", name=f"b{nm}{ft}")
                nc.sync.dma_start(out=t[:], in_=dram[dr][ft])
                tiles.append(t)
            bpair[nm] = tiles
        # ones columns of V' (memset; bitcast f32r views to f32)
        for s in range(4):
            col64 = v_t[s][:, :, 64:65]
            if SLOT_DT[s] == F32R:
                col64 = col64.bitcast(F32)
            nc.vector.memset(col64, 1.0)

        # ---- constants (needed from attention onward) ----
        mask_a_sb, mask_b_sb = [], []
        for mi in range(2):
            t = consts.tile([128, 64], F32, tag=f"maska{mi}", name=f"maska{mi}")
            nc.sync.dma_start(out=t[:], in_=dram["masks_a"][mi])
            mask_a_sb.append(t)
            t = consts.tile([128, 256], F32, tag=f"maskb{mi}", name=f"maskb{mi}")
            nc.sync.dma_start(out=t[:], in_=dram["masks_b"][mi])
            mask_b_sb.append(t)
        nbtot = sum(len(slot_offsets(s)) for s in range(4))
        bias_tile = consts.tile([128, nbtot], F32, tag="bias_all", name="bias_all")
        nc.sync.dma_start(out=bias_tile[:], in_=dram["bias_all"])
        bias_sb = []
        col = 0
        for s in range(4):
            d = {}
            for o in slot_offsets(s):
                d[o] = bias_tile[:, col:col + 1]
                col += 1
            bias_sb.append(d)
        wout_sb = []
        for nm in ("wout_ab", "wout_cd"):
            t = consts.tile([128, D], F32R, tag=nm, name=nm)
            nc.sync.dma_start(out=t[:], in_=dram[nm])
            wout_sb.append(t)
        yconst_bc = consts.tile([128, D], F32, tag="yconst_bc", name="yconst_bc")
        yconst_bcast = bass.AP(
            tensor=dram["yconst"].tensor, offset=0, ap=[[0, 128], [1, D]])
        nc.sync.dma_start(out=yconst_bc[:], in_=yconst_bcast)

        by_slot = []
        for s in range(4):
            by_it = {}
            for it, jt, o in slot_blocks(s):
                by_it.setdefault(it, []).append((jt, o))
            by_slot.append(by_it)

        def emit_proj(ch):
            """Load x^T chunk ch and project q/k/v for its 512 tokens."""
            xt = []
            for kt in range(8):
                t = xtp.tile([128, 512], F32R, tag="xt", name="xt")
                nc.scalar.dma_start(
                    out=t[:],
                    in_=dram["xT"][kt * 128:(kt + 1) * 128, ch * 512:(ch + 1) * 512])
                xt.append(t)
            sl = slice(ch * 512, (ch + 1) * 512)
            for nm, dst in (("q", q_p), ("k", k_p)):
                for ft in range(2):      # feature pair (slots 2ft, 2ft+1)
                    ps = big_ps.tile([128, 512], F32, tag="big", name="qkv")
                    for kt in range(8):
                        nc.tensor.matmul(
                            ps[:], w_sb[nm][kt][:, ft * 128:(ft + 1) * 128],
                            xt[kt][:], start=(kt == 0), stop=(kt == 7))
                    nc.vector.tensor_scalar_add(
                        dst[ft][:, sl], ps[:], bpair[nm][ft][:])
            for tl in range(4):
                tt = ch * 4 + tl
                ps = big_ps.tile([128, 512], F32, tag="big", name="qkvv")
                for kt in range(8):
                    nc.tensor.matmul(
                        ps[:, 0:256], xt[kt][:, tl * 128:(tl + 1) * 128],
                        w_sb["v"][kt][:], start=(kt == 0), stop=(kt == 7))
                for s in range(4):
                    nc.vector.tensor_copy(
                        v_t[s][:, tt:tt + 1, 0:64], ps[:, s * 64:(s + 1) * 64])

        def emit_scores(s, it):
            """Scores+mask+exp for one chunk; returns probs list."""
            W, dt_s = SLOT_W[s], SLOT_DT[s]
            prs = []
            h0 = (s % 2) * 64
            kp_s = k_p[s // 2]
            qp_s = q_p[s // 2]
            for jt, o in by_slot[s][it]:
                sc = sc_ps.tile([128, W], F32, tag="sc", name="sc")
                nc.tensor.matmul(
                    sc[:], kp_s[h0:h0 + 64, jt * 128:(jt + 1) * 128],
                    qp_s[h0:h0 + 64, it * W:(it + 1) * W],
                    start=True, stop=True)
                if o <= 127:  # diagonal block -> causal mask add
                    if s == 0:
                        msk = mask_a_sb[o // 64]
                    else:
                        msk = mask_b_sb[0 if o == 0 else 1]
                    nc.vector.tensor_add(sc[:], sc[:], msk[:])
                ptag = "pr_a" if s == 0 else "pr_b"
                pr = prp.tile([128, W], dt_s, tag=ptag, name="pr", bufs=(12 if s == 0 else 20))
                nc.scalar.activation(pr[:], sc[:], Exp, bias=bias_sb[s][o][:])
                prs.append((jt, pr))
            return prs

        def emit_pv(s, it, prs):
            """PV accumulation + normalize epilogue for one chunk."""
            W = SLOT_W[s]
            pv = pv_ps.tile([65, W], F32, tag="pv", name="pv")
            for bi, (jt, pr) in enumerate(prs):
                nc.tensor.matmul(
                    pv[:], v_t[s][:, jt:jt + 1, :], pr[:],
                    start=(bi == 0), stop=(bi == len(prs) - 1))
            rr = lp.tile([1, W], F32, tag="rr", name="rr")
            nc.vector.reciprocal(rr[:], pv[64:65, :])
            rb = rbp.tile([64, W], F32, tag="rb", name="rb")
            nc.gpsimd.partition_broadcast(rb[:], rr[:])
            dst = attn_sb[s // 2]
            r0 = (s % 2) * 64
            nc.vector.tensor_mul(
                dst[r0:r0 + 64, it * W:(it + 1) * W], pv[0:64, :], rb[:])

        def emit_yproj(tt):
            """Out-projection for token tile tt (needs attn rows complete)."""
            for oc in range(2):
                py = big_ps.tile([128, 512], F32, tag="big", name="py")
                nc.tensor.matmul(
                    py[:], attn_sb[0][:, tt * 128:(tt + 1) * 128],
                    wout_sb[0][:, oc * 512:(oc + 1) * 512],
                    start=True, stop=False)
                nc.tensor.matmul(
                    py[:], attn_sb[1][:, tt * 128:(tt + 1) * 128],
                    wout_sb[1][:, oc * 512:(oc + 1) * 512],
                    start=False, stop=True)
                ysb = yp.tile([128, 512], F32, tag="ysb", name="ysb")
                nc.vector.tensor_add(ysb[:], py[:], yconst_bc[:, oc * 512:(oc + 1) * 512])
                nc.scalar.dma_start(
                    out=y_out[tt * 128:(tt + 1) * 128, oc * 512:(oc + 1) * 512],
                    in_=ysb[:])

        # ---- fused schedule: per 512-token chunk: project -> attention -> yproj
        # software pipeline: scores of chunk i emitted before pv of chunk i-1
        prev = None
        pending_y = []
        for ch in range(4):
            emit_proj(ch)
            a0 = ch * 8          # slot-A chunks in this ch (W=64): a0..a0+7
            b0 = ch * 2          # slot B/C/D chunks in this ch (W=256): b0, b0+1
            chunks = [
                (2, b0), (0, a0), (0, a0 + 1),
                (3, b0), (0, a0 + 2), (0, a0 + 3),
                (1, b0), (0, a0 + 4),
                (2, b0 + 1), (0, a0 + 5),
                (3, b0 + 1), (0, a0 + 6),
                (1, b0 + 1), (0, a0 + 7),
            ]
            for s, it in chunks:
                cur = (s, it, emit_scores(s, it))
                if prev is not None:
                    emit_pv(*prev)
                prev = cur
            # out-proj for the PREVIOUS chunk's tokens (its attn rows are done
            # once this chunk's pv for all slots completes; emit with 1-chunk lag)
            if ch > 0:
                for tt in range((ch - 1) * 4, ch * 4):
                    pending_y.append(tt)
            while len(pending_y) > 2:
                emit_yproj(pending_y.pop(0))
        emit_pv(*prev)
        for tt in pending_y + list(range(12, 16)):
            emit_yproj(tt)


def make_in_maps(x, w_qkv, b_qkv, w_out, b_out):
    """Host-side sharding + constant prep. Returns list of 8 in_maps."""
    x = np.asarray(x, np.float32)
    w_qkv = np.asarray(w_qkv, np.float32)
    b_qkv = np.asarray(b_qkv, np.float32)
    w_out = np.asarray(w_out, np.float32)
    b_out = np.asarray(b_out, np.float32)

    slopes = (2.0 ** (-(np.arange(1, H + 1)) * 8.0 / H)).astype(np.float64)

    # shared constants
    masks_a = np.empty((2, 128, 64), np.float32)
    for mi, o in enumerate((0, 64)):
        p = np.arange(128)[:, None]
        f = np.arange(64)[None, :]
        masks_a[mi] = np.where(p <= o + f, 0.0, NEG)
    masks_b = np.empty((2, 128, 256), np.float32)
    for mi, o in enumerate((0, -128)):
        p = np.arange(128)[:, None]
        f = np.arange(256)[None, :]
        masks_b[mi] = np.where(p <= o + f, 0.0, NEG)

    in_maps = []
    for c in range(N_CORES):
        b, j = divmod(c, 4)
        heads = [j, j + 4, j + 8, j + 12]
        cols = np.concatenate([np.arange(h * HD, (h + 1) * HD) for h in heads])
        wq = w_qkv[cols, :] / 8.0                  # [256, 1024], scale folded
        wk = w_qkv[D + cols, :]
        wv = w_qkv[2 * D + cols, :]
        bq = b_qkv[cols] / 8.0
        bk = b_qkv[D + cols]
        bv = b_qkv[2 * D + cols]
        w_out_loc = w_out[:, cols]                  # [1024, 256]
        yconst = (w_out_loc @ bv + b_out / 4.0).astype(np.float32)[None, :]

        cols = []
        for s in range(4):
            Wl = SLOT_W[s]
            sl = slopes[heads[s]]
            for o in slot_offsets(s):
                cols.append(sl * (np.arange(128) - o - Wl + 1))
        bias_all = np.stack(cols, axis=1).astype(np.float32)

        in_maps.append(dict(
            xT=np.ascontiguousarray(x[b].T),
            wqT=np.ascontiguousarray(wq.T),
            wkT=np.ascontiguousarray(wk.T),
            wvT=np.ascontiguousarray(wv.T),
            bq_p=np.ascontiguousarray(bq.reshape(2, 128, 1)),
            bk_p=np.ascontiguousarray(bk.reshape(2, 128, 1)),
            masks_a=masks_a, masks_b=masks_b, bias_all=bias_all,
            wout_ab=np.ascontiguousarray(w_out_loc[:, 0:128].T),
            wout_cd=np.ascontiguousarray(w_out_loc[:, 128:256].T),
            yconst=yconst,
        ))
    return in_maps


_NC_CACHE = {}


def _get_nc(repeat=1):
    if repeat not in _NC_CACHE:
        _NC_CACHE[repeat] = build_nc(repeat)
    return _NC_CACHE[repeat]


def kernel(x, w_qkv, b_qkv, w_out, b_out, block_mask=None):
    in_maps = make_in_maps(x, w_qkv, b_qkv, w_out, b_out)
    nc = _get_nc(1)
    res = run_bass_kernel_spmd(nc, in_maps, list(range(N_CORES)), trace=False)
    y = np.zeros((B, S, D), np.float64)
    for c in range(N_CORES):
        y[c // 4] += res.results[c]["y"].astype(np.float64)
    return y.astype(np.float32)
